# revision 10
# baseline (speedup 1.0000x reference)
"""Trainium2 Bass kernel for nn_ClusteringLayer (vq_codebook).

Math (ALPHA=1 so the power() is the identity):
    dist2[r, j] = |x_r|^2 + |c_j|^2 - 2 x_r . c_j
    q = 1 / (1 + dist2)
    q = q / sum_j q                        (row normalize)

Sharding: data-parallel over 8 cores, 8192 rows each; clusters replicated.

Per-core pipeline (per 128-row chunk, DMA batched 4 chunks/transfer):
    DMA   x block [128, 4, 256]
    ACT   Square+accum        -> x2 [128, 1] per chunk
    PE    2x transpose        -> xT in PSUM; evacuate PSUM->SBUF (ACT/DVE)
    PE    f32r matmuls: xT.T @ (-2 cT)  (K=256, 2 passes)
          + ones-row x (|c|^2 + 1)      (K=1 augmentation)
          => PSUM = (c2 + 1) - 2 G
    ACT   Relu(psum + bias=x2) -> u = 1 + dist2  (> 0 always)
    DVE   QUAD_RECIP_REDUCE: q_un ~= 1/u AND s = rowsum(q_un), one pass
          (bitwise-NOT exponent-flip seed + deg-2 minimax, ~5e-5 rel err)
    DVE   s_inv = 1/s (exact); out = q_un * s_inv
    DMA   q block out [128, 4, 256]
"""

import os
import numpy as np

import concourse.bacc as bacc
import concourse.tile as tile
import concourse.mybir as mybir
from concourse.bass_utils import run_bass_kernel_spmd
from concourse.masks import make_identity

N_CORES = 8
N, D, K = 65536, 256, 256
NS = N // N_CORES          # rows per core
P = 128
NCHUNK = NS // P           # 64 chunks of 128 rows

F32 = mybir.dt.float32
F32R = mybir.dt.float32r
AF = mybir.ActivationFunctionType

# ---------------------------------------------------------------------------
# Custom DVE op: approximate reciprocal (deg-2 minimax on the folded interval)
# fused with a free-axis sum.  body depth 7 + accum stage 8 = fits the 8-slice
# DVE pipe, unlike RECIPROCAL_APPROX_FAST (8-stage body, no room for accum).
#   n = bitcast(~u);  t = u*n  in [-4.5, -4];  1/u = n * (1/t)
#   1/t ~= A + B t + C t^2   (minimax, 5.1e-5 relative)
# ---------------------------------------------------------------------------
QR_A, QR_B, QR_C = -0.7071065, -0.16652197, -0.013060541


def _register_quad_recip():
    from operator import add
    import concourse.dve_ops as dve_ops
    from concourse.dve_spec import AluOp, Bin, Src0, Zero, C0, C1, C2, Spec, lower
    from concourse.dve_spec import _has_src1 as has_src1
    from concourse.dve_uop import DveOpSpec

    name = "QUAD_RECIP_REDUCE_CLK"
    if name in dve_ops._SUB_OPCODE_FOR_NAME:
        return next(op for op in dve_ops.OPS if op.name == name)

    _n = Bin(AluOp.BITWISE_NOT, Src0, Src0)
    _t = Src0 * _n
    body = _n * (C0 + _t * (C1 + _t * C2))

    def _ref(in0, in1, s0, s1, imm2):
        u = in0.astype(np.float32)
        n = (~u.view(np.int32)).view(np.float32)
        t = (u * n).astype(np.float32)
        h = (np.float32(imm2) * t).astype(np.float32)
        h = (h + np.float32(s1)).astype(np.float32)
        h = (h * t).astype(np.float32)
        h = (h + np.float32(s0)).astype(np.float32)
        b = (n * h).astype(np.float32)
        return b, b.reshape(b.shape[0], -1).sum(axis=-1, keepdims=True)

    spec = Spec(body=body, accum=add, accum_init=Zero, reference=_ref)

    row = dve_ops._CUSTOM_DVE_ROW_BASE + len(dve_ops.OPS)
    shas = {}
    for ver in ("v3", "v4"):
        try:
            shas[ver] = DveOpSpec(
                name=name, opcode=row, uops=lower(spec, ver=ver),
                rd1_en=has_src1(spec),
            ).sha(ver)
        except Exception:
            pass

    op = dve_ops.DveOp(name, spec, subdim=False, uops_sha=shas)
    dve_ops.OPS.append(op)
    dve_ops._SUB_OPCODE_FOR_NAME[name] = row
    dve_ops.CUSTOM_DVE_SPECS[name] = spec
    return op


def build(reps: int = 1, mm_dtype: str = "f32r", recip: str = "quad",
          evac: str = "split", bufs: int = 6, psum_bufs: int = 4,
          dma_batch: int = 4):
    B = dma_batch
    NBLK = NCHUNK // B
    nc = bacc.Bacc(target_bir_lowering=False)
    MMDT = F32R if mm_dtype == "f32r" else F32
    x = nc.dram_tensor("x", [NS, D], F32, kind="ExternalInput")
    cm2t = nc.dram_tensor("cm2t", [D, K], MMDT, kind="ExternalInput")
    c2p1 = nc.dram_tensor("c2p1", [1, K], MMDT, kind="ExternalInput")
    onesrow = nc.dram_tensor("onesrow", [1, P], MMDT, kind="ExternalInput")
    q = nc.dram_tensor("q", [NS, K], F32, kind="ExternalOutput")

    # block n, chunk b, partition p: row = (n*B + b)*128 + p
    xr = x.ap().rearrange("(n b p) d -> n p b d", b=B, p=P)
    qr = q.ap().rearrange("(n b p) j -> n p b j", b=B, p=P)

    qrop = _register_quad_recip() if recip == "quad" else None

    with tile.TileContext(nc) as tc:
        with (
            tc.tile_pool(name="const", bufs=1) as const,
            tc.tile_pool(name="xio", bufs=3) as xio,
            tc.tile_pool(name="mid", bufs=bufs) as mid,
            tc.tile_pool(name="small", bufs=2 * bufs) as small,
            tc.tile_pool(name="oio", bufs=3) as oio,
            # pst tiles are 2 banks each (2B*128 f32), psd tiles 1 bank;
            # 2*2 + 4*1 = 8 banks exactly
            tc.tile_pool(name="pst", bufs=2, space="PSUM") as pst,
            tc.tile_pool(name="psd", bufs=psum_bufs, space="PSUM") as psd,
        ):
            cm2t_sb = const.tile([P, 2, K], MMDT)
            nc.sync.dma_start(cm2t_sb, cm2t.ap().rearrange("(o p) j -> p o j", p=P))
            c2p1_sb = const.tile([1, K], MMDT)
            nc.sync.dma_start(c2p1_sb, c2p1.ap())
            ones_sb = const.tile([1, P], MMDT)
            nc.sync.dma_start(ones_sb, onesrow.ap())
            ident = const.tile([P, P], F32)
            make_identity(nc, ident)

            for _ in range(reps):
                for blk in range(NCHUNK // B):
                    x_blk = xio.tile([P, B, D], F32, tag="x_blk")
                    nc.sync.dma_start(x_blk, xr[blk])
                    out_blk = oio.tile([P, B, K], F32, tag="out_blk")

                    # whole-block transpose: 2B matmul-transposes into one
                    # PSUM tile, then a single batched PSUM->SBUF evacuation
                    xt_ps = pst.tile([P, 2 * B, P], F32, tag="xt_ps")
                    for b in range(B):
                        nc.tensor.transpose(xt_ps[:, 2 * b + 0],
                                            x_blk[:, b, 0:P], ident)
                        nc.tensor.transpose(xt_ps[:, 2 * b + 1],
                                            x_blk[:, b, P:D], ident)
                    xt_sb = mid.tile([P, 2 * B, P], MMDT, tag="xt_sb")
                    if evac == "any":
                        nc.any.tensor_copy(out=xt_sb, in_=xt_ps)
                    elif evac == "scalar":
                        nc.scalar.copy(xt_sb, xt_ps)
                    elif evac == "split":
                        h = B  # first half of the slots on ACT, rest on DVE
                        nc.scalar.copy(xt_sb[:, :h], xt_ps[:, :h])
                        nc.vector.tensor_copy(xt_sb[:, h:], xt_ps[:, h:])
                    else:
                        nc.vector.tensor_copy(xt_sb, xt_ps)

                    s_blk = small.tile([P, B], F32, tag="s_blk")
                    sinv_blk = small.tile([P, B], F32, tag="sinv_blk")

                    for b in range(B):
                        x_t = x_blk[:, b, :]

                        sq_scr = mid.tile([P, D], F32, tag="sq_scr")
                        x2 = small.tile([P, 1], F32, tag="x2")
                        nc.scalar.activation(sq_scr, x_t, AF.Square,
                                             accum_out=x2)

                        dist_ps = psd.tile([P, K], F32, tag="dist_ps")
                        nc.tensor.matmul(dist_ps, xt_sb[:, 2 * b + 0],
                                         cm2t_sb[:, 0], start=True, stop=False)
                        nc.tensor.matmul(dist_ps, xt_sb[:, 2 * b + 1],
                                         cm2t_sb[:, 1], start=False, stop=False)
                        nc.tensor.matmul(dist_ps, ones_sb, c2p1_sb,
                                         start=False, stop=True)

                        # u = 1 + dist2 (Relu == identity: u > 0)
                        u_sb = mid.tile([P, K], F32, tag="u_sb")
                        nc.scalar.activation(u_sb, dist_ps, AF.Relu, bias=x2)

                        q_un = mid.tile([P, K], F32, tag="q_un")
                        if recip == "quad":
                            nc.vector._custom_dve(
                                qrop, out=q_un, in0=u_sb,
                                s0=QR_A, s1=QR_B, imm2=QR_C,
                                accum_out=s_blk[:, b:b + 1],
                            )
                        else:
                            if recip == "fast":
                                nc.vector.reciprocal_approx_fast(out=q_un,
                                                                 in_=u_sb)
                            elif recip == "accurate":
                                scr = mid.tile([P, K], F32, tag="recip_scr")
                                nc.vector.reciprocal_approx_accurate(
                                    out=q_un, in_=u_sb, scratch=scr)
                            else:
                                nc.vector.reciprocal(out=q_un, in_=u_sb)
                            nc.vector.tensor_reduce(
                                s_blk[:, b:b + 1], q_un,
                                axis=mybir.AxisListType.X,
                                op=mybir.AluOpType.add)

                        nc.vector.reciprocal(sinv_blk[:, b:b + 1],
                                             s_blk[:, b:b + 1])
                        nc.vector.tensor_scalar_mul(out_blk[:, b, :], q_un,
                                                    sinv_blk[:, b:b + 1])

                    nc.sync.dma_start(qr[blk], out_blk)

    nc.compile()
    return nc


_BUILD_CACHE: dict = {}


def _get_nc(**kw):
    key = tuple(sorted(kw.items()))
    if key not in _BUILD_CACHE:
        _BUILD_CACHE[key] = build(**kw)
    return _BUILD_CACHE[key]


def prep_inputs(x: np.ndarray, clusters: np.ndarray):
    x = np.ascontiguousarray(np.asarray(x, dtype=np.float32))
    clusters = np.asarray(clusters, dtype=np.float32)
    cm2t = np.ascontiguousarray(-2.0 * clusters.T)
    c2 = (clusters * clusters).sum(axis=1, dtype=np.float32)
    c2p1 = np.ascontiguousarray((c2 + np.float32(1.0))[None, :])
    ones = np.ones((1, P), dtype=np.float32)
    shards = np.split(x, N_CORES, axis=0)
    return [{"x": s, "cm2t": cm2t, "c2p1": c2p1, "onesrow": ones}
            for s in shards]


def run(nc, in_maps):
    res = run_bass_kernel_spmd(nc, in_maps, core_ids=list(range(N_CORES)))
    return np.concatenate([r["q"] for r in res.results], axis=0)


def kernel(x: np.ndarray, clusters: np.ndarray) -> np.ndarray:
    in_maps = prep_inputs(x, clusters)
    nc = _get_nc(
        reps=1,
        mm_dtype=os.environ.get("CLK_MM_DTYPE", "f32r"),
        recip=os.environ.get("CLK_RECIP", "quad"),
        evac=os.environ.get("CLK_EVAC", "split"),
    )
    return run(nc, in_maps)


# revision 15
# speedup vs baseline: 365.4165x; 365.4165x over previous
"""Trainium2 Bass kernel for nn_ClusteringLayer (vq_codebook).

Math (ALPHA=1 so the power() is the identity):
    dist2[r, j] = |x_r|^2 + |c_j|^2 - 2 x_r . c_j
    q = 1 / (1 + dist2)
    q = q / sum_j q                        (row normalize)

Sharding: data-parallel over 8 cores, 8192 rows each; clusters replicated.

Per-core pipeline (per 128-row chunk, DMA batched 4 chunks/transfer):
    DMA   x block [128, 4, 256]
    ACT   Square+accum        -> x2 [128, 1] per chunk
    PE    2x transpose        -> xT in PSUM; evacuate PSUM->SBUF (ACT/DVE)
    PE    f32r matmuls: xT.T @ (-2 cT)  (K=256, 2 passes)
          + ones-row x (|c|^2 + 1)      (K=1 augmentation)
          => PSUM = (c2 + 1) - 2 G
    ACT   Relu(psum + bias=x2) -> u = 1 + dist2  (> 0 always)
    DVE   QUAD_RECIP_REDUCE: q_un ~= 1/u AND s = rowsum(q_un), one pass
          (bitwise-NOT exponent-flip seed + deg-2 minimax, ~5e-5 rel err)
    DVE   s_inv = 1/s (exact); out = q_un * s_inv
    DMA   q block out [128, 4, 256]
"""

import os
import numpy as np

import concourse.bacc as bacc
import concourse.tile as tile
import concourse.mybir as mybir
from concourse.bass_utils import run_bass_kernel_spmd
from concourse.masks import make_identity

N_CORES = 8
N, D, K = 65536, 256, 256
NS = N // N_CORES          # rows per core
P = 128
NCHUNK = NS // P           # 64 chunks of 128 rows

F32 = mybir.dt.float32
F32R = mybir.dt.float32r
AF = mybir.ActivationFunctionType

# ---------------------------------------------------------------------------
# Custom DVE op: approximate reciprocal (deg-2 minimax on the folded interval)
# fused with a free-axis sum.  body depth 7 + accum stage 8 = fits the 8-slice
# DVE pipe, unlike RECIPROCAL_APPROX_FAST (8-stage body, no room for accum).
#   n = bitcast(~u);  t = u*n  in [-4.5, -4];  1/u = n * (1/t)
#   1/t ~= A + B t + C t^2   (minimax, 5.1e-5 relative)
# ---------------------------------------------------------------------------
QR_A, QR_B, QR_C = -0.7071065, -0.16652197, -0.013060541


def _register_dve_op(name, spec):
    import concourse.dve_ops as dve_ops
    from concourse.dve_spec import _has_src1 as has_src1, lower
    from concourse.dve_uop import DveOpSpec

    if name in dve_ops._SUB_OPCODE_FOR_NAME:
        return next(op for op in dve_ops.OPS if op.name == name)
    row = dve_ops._CUSTOM_DVE_ROW_BASE + len(dve_ops.OPS)
    shas = {}
    for ver in ("v3", "v4"):
        try:
            shas[ver] = DveOpSpec(
                name=name, opcode=row, uops=lower(spec, ver=ver),
                rd1_en=has_src1(spec),
            ).sha(ver)
        except Exception:
            pass
    op = dve_ops.DveOp(name, spec, subdim=False, uops_sha=shas)
    dve_ops.OPS.append(op)
    dve_ops._SUB_OPCODE_FOR_NAME[name] = row
    dve_ops.CUSTOM_DVE_SPECS[name] = spec
    return op


def _register_quad_recip():
    from operator import add
    from concourse.dve_spec import AluOp, Bin, Src0, Zero, C0, C1, C2, Spec

    _n = Bin(AluOp.BITWISE_NOT, Src0, Src0)
    _t = Src0 * _n
    body = _n * (C0 + _t * (C1 + _t * C2))

    def _ref(in0, in1, s0, s1, imm2):
        u = in0.astype(np.float32)
        n = (~u.view(np.int32)).view(np.float32)
        t = (u * n).astype(np.float32)
        h = (np.float32(imm2) * t).astype(np.float32)
        h = (h + np.float32(s1)).astype(np.float32)
        h = (h * t).astype(np.float32)
        h = (h + np.float32(s0)).astype(np.float32)
        b = (n * h).astype(np.float32)
        return b, b.reshape(b.shape[0], -1).sum(axis=-1, keepdims=True)

    return _register_dve_op(
        "QUAD_RECIP_REDUCE_CLK",
        Spec(body=body, accum=add, accum_init=Zero, reference=_ref))


def _register_square_reduce():
    from operator import add
    from concourse.dve_spec import Src0, Zero, Spec, sq

    def _ref(in0, in1, s0, s1, imm2):
        b = (in0.astype(np.float32) * in0.astype(np.float32)).astype(np.float32)
        return b, b.reshape(b.shape[0], -1).sum(axis=-1, keepdims=True)

    return _register_dve_op(
        "SQUARE_REDUCE_CLK",
        Spec(body=sq(Src0), accum=add, accum_init=Zero, reference=_ref))


def build(reps: int = 1, mm_dtype: str = "f32r", recip: str = "quad",
          evac: str = "split", bufs: int = 6, psum_bufs: int = 4,
          dma_batch: int = 4, x2eng: str = "act"):
    B = dma_batch
    NBLK = NCHUNK // B
    nc = bacc.Bacc(target_bir_lowering=False)
    MMDT = F32R if mm_dtype == "f32r" else F32
    x = nc.dram_tensor("x", [NS, D], F32, kind="ExternalInput")
    cm2t = nc.dram_tensor("cm2t", [D, K], MMDT, kind="ExternalInput")
    c2p1 = nc.dram_tensor("c2p1", [1, K], MMDT, kind="ExternalInput")
    onesrow = nc.dram_tensor("onesrow", [1, P], MMDT, kind="ExternalInput")
    q = nc.dram_tensor("q", [NS, K], F32, kind="ExternalOutput")

    # block n, chunk b, partition p: row = (n*B + b)*128 + p
    xr = x.ap().rearrange("(n b p) d -> n p b d", b=B, p=P)
    qr = q.ap().rearrange("(n b p) j -> n p b j", b=B, p=P)

    qrop = _register_quad_recip() if recip == "quad" else None
    sqop = _register_square_reduce() if x2eng == "dve" else None

    with tile.TileContext(nc) as tc:
        with (
            tc.tile_pool(name="const", bufs=1) as const,
            tc.tile_pool(name="xio", bufs=3) as xio,
            tc.tile_pool(name="mid", bufs=bufs) as mid,
            tc.tile_pool(name="small", bufs=2 * bufs) as small,
            tc.tile_pool(name="oio", bufs=3) as oio,
            # pst tiles are 2 banks each (2B*128 f32), psd tiles 1 bank;
            # 2*2 + 4*1 = 8 banks exactly
            tc.tile_pool(name="pst", bufs=2, space="PSUM") as pst,
            tc.tile_pool(name="psd", bufs=psum_bufs, space="PSUM") as psd,
        ):
            cm2t_sb = const.tile([P, 2, K], MMDT)
            nc.sync.dma_start(cm2t_sb, cm2t.ap().rearrange("(o p) j -> p o j", p=P))
            c2p1_sb = const.tile([1, K], MMDT)
            nc.sync.dma_start(c2p1_sb, c2p1.ap())
            ones_sb = const.tile([1, P], MMDT)
            nc.sync.dma_start(ones_sb, onesrow.ap())
            ident = const.tile([P, P], F32)
            make_identity(nc, ident)

            import contextlib

            rep_ctx = (tc.For_i(0, reps, 1) if reps > 1
                       else contextlib.nullcontext())
            with rep_ctx:
                for blk in range(NCHUNK // B):
                    x_blk = xio.tile([P, B, D], F32, tag="x_blk")
                    nc.sync.dma_start(x_blk, xr[blk])
                    out_blk = oio.tile([P, B, K], F32, tag="out_blk")

                    # whole-block transpose: 2B matmul-transposes into one
                    # PSUM tile, then a single batched PSUM->SBUF evacuation
                    xt_ps = pst.tile([P, 2 * B, P], F32, tag="xt_ps")
                    for b in range(B):
                        nc.tensor.transpose(xt_ps[:, 2 * b + 0],
                                            x_blk[:, b, 0:P], ident)
                        nc.tensor.transpose(xt_ps[:, 2 * b + 1],
                                            x_blk[:, b, P:D], ident)
                    xt_sb = mid.tile([P, 2 * B, P], MMDT, tag="xt_sb")
                    if evac == "any":
                        nc.any.tensor_copy(out=xt_sb, in_=xt_ps)
                    elif evac == "scalar":
                        nc.scalar.copy(xt_sb, xt_ps)
                    elif evac == "split":
                        h = B  # first half of the slots on ACT, rest on DVE
                        nc.scalar.copy(xt_sb[:, :h], xt_ps[:, :h])
                        nc.vector.tensor_copy(xt_sb[:, h:], xt_ps[:, h:])
                    else:
                        nc.vector.tensor_copy(xt_sb, xt_ps)

                    s_blk = small.tile([P, B], F32, tag="s_blk")
                    sinv_blk = small.tile([P, B], F32, tag="sinv_blk")

                    for b in range(B):
                        x_t = x_blk[:, b, :]

                        sq_scr = mid.tile([P, D], F32, tag="sq_scr")
                        x2 = small.tile([P, 1], F32, tag="x2")
                        if x2eng == "dve":
                            nc.vector._custom_dve(sqop, out=sq_scr, in0=x_t,
                                                  accum_out=x2)
                        else:
                            nc.scalar.activation(sq_scr, x_t, AF.Square,
                                                 accum_out=x2)

                        dist_ps = psd.tile([P, K], F32, tag="dist_ps")
                        nc.tensor.matmul(dist_ps, xt_sb[:, 2 * b + 0],
                                         cm2t_sb[:, 0], start=True, stop=False)
                        nc.tensor.matmul(dist_ps, xt_sb[:, 2 * b + 1],
                                         cm2t_sb[:, 1], start=False, stop=False)
                        nc.tensor.matmul(dist_ps, ones_sb, c2p1_sb,
                                         start=False, stop=True)

                        # u = 1 + dist2 (Relu == identity: u > 0)
                        u_sb = mid.tile([P, K], F32, tag="u_sb")
                        nc.scalar.activation(u_sb, dist_ps, AF.Relu, bias=x2)

                        q_un = mid.tile([P, K], F32, tag="q_un")
                        if recip == "quad":
                            nc.vector._custom_dve(
                                qrop, out=q_un, in0=u_sb,
                                s0=QR_A, s1=QR_B, imm2=QR_C,
                                accum_out=s_blk[:, b:b + 1],
                            )
                        else:
                            if recip == "fast":
                                nc.vector.reciprocal_approx_fast(out=q_un,
                                                                 in_=u_sb)
                            elif recip == "accurate":
                                scr = mid.tile([P, K], F32, tag="recip_scr")
                                nc.vector.reciprocal_approx_accurate(
                                    out=q_un, in_=u_sb, scratch=scr)
                            else:
                                nc.vector.reciprocal(out=q_un, in_=u_sb)
                            nc.vector.tensor_reduce(
                                s_blk[:, b:b + 1], q_un,
                                axis=mybir.AxisListType.X,
                                op=mybir.AluOpType.add)

                        nc.vector.reciprocal(sinv_blk[:, b:b + 1],
                                             s_blk[:, b:b + 1])
                        nc.vector.tensor_scalar_mul(out_blk[:, b, :], q_un,
                                                    sinv_blk[:, b:b + 1])

                    nc.sync.dma_start(qr[blk], out_blk)

    nc.compile()
    return nc


_BUILD_CACHE: dict = {}


def _get_nc(**kw):
    key = tuple(sorted(kw.items()))
    if key not in _BUILD_CACHE:
        _BUILD_CACHE[key] = build(**kw)
    return _BUILD_CACHE[key]


def prep_inputs(x: np.ndarray, clusters: np.ndarray):
    x = np.ascontiguousarray(np.asarray(x, dtype=np.float32))
    clusters = np.asarray(clusters, dtype=np.float32)
    cm2t = np.ascontiguousarray(-2.0 * clusters.T)
    c2 = (clusters * clusters).sum(axis=1, dtype=np.float32)
    c2p1 = np.ascontiguousarray((c2 + np.float32(1.0))[None, :])
    ones = np.ones((1, P), dtype=np.float32)
    shards = np.split(x, N_CORES, axis=0)
    return [{"x": s, "cm2t": cm2t, "c2p1": c2p1, "onesrow": ones}
            for s in shards]


def run(nc, in_maps):
    res = run_bass_kernel_spmd(nc, in_maps, core_ids=list(range(N_CORES)))
    return np.concatenate([r["q"] for r in res.results], axis=0)


def kernel(x: np.ndarray, clusters: np.ndarray) -> np.ndarray:
    in_maps = prep_inputs(x, clusters)
    nc = _get_nc(
        reps=1,
        mm_dtype=os.environ.get("CLK_MM_DTYPE", "f32r"),
        recip=os.environ.get("CLK_RECIP", "quad"),
        evac=os.environ.get("CLK_EVAC", "split"),
    )
    return run(nc, in_maps)
